# revision 29
# baseline (speedup 1.0000x reference)
# CopyGenerator kernel for 8 TRN2 NeuronCores (Bass/Tile, SPMD) — v5.
#
# reference computation:
#   logits = hidden @ W.T + b                      [B=1024, V=50000]
#   ml = logits with col COPY(4) = 1e-10
#   prob = softmax(ml); copy = sigmoid(logits[:, 4])
#   out_prob = prob*(1-copy); out_prob[b, alignment[src[b,s]]] += attn[b,s]*copy[b]
#   out_prob[:, 0] = EPS; norm = out_prob.sum(-1)
#   out = log(out_prob/norm + EPS)
#
# Strategy (tensor-parallel over vocab, VC=6250 cols/core): for the
# ~49.9k/50k columns with no scatter contribution,
#   out[b,v] = ml[b,v] + C[b],  C = ln((1-copy)/(se*norm))
# exactly (log-domain identity; inner +EPS is negligible: norm-rel ~9e-5,
# validated in simcheck.py).  Device per core: fp8 DoubleRow matmul ->
# DVE adds bias & casts bf16 (1024-wide PSUM pairs) -> bf16 out DMA per
# pair -> ACT exp in 2 halves per row (accum -> per-row partial softmax
# sum) -> ships a tiny [128,3,8] per-row stats tensor.  The host sums the
# 8 cores' partial stats (24KB total), forms C, and adds it during the
# bf16->fp32 conversion, then patches the <=128 scattered columns per row
# + PAD/COPY columns.  No collectives: on this axon setup the entry
# barrier + AllReduce cost 40-60us (cores start skewed), which starved
# the in-order DVE queue and bunched all output DMA into the tail.
#
# Per-core HBM: W 6.8MB (resident) + ht 1MB + bias 1.6MB + out 12.8MB
# bf16 ~= 22MB; TensorE (~96us fp8 DR matmul) is the design bottleneck.
import numpy as np
import ml_dtypes

import concourse.bacc as bacc
import concourse.bass as bass
import concourse.mybir as mybir
import concourse.tile as tile
from concourse import bass_utils

FP32 = mybir.dt.float32
BF16 = mybir.dt.bfloat16
FP8 = mybir.dt.float8e4
AF = mybir.ActivationFunctionType
ALU = mybir.AluOpType

B, S, H, V = 1024, 128, 1024, 50000
NCORES = 8
VC = V // NCORES          # 6250 vocab columns per core
NBT = B // 128            # 8 batch tiles of 128 rows
KD = 4                    # 4 DoubleRow chunks of K=256
COPY, PAD, EPS = 4, 0, 1e-10

CHUNK = 512               # W-DMA granularity (last chunk zero-padded)
NWCH = 13                 # 12x512 + 106
PAIR = 1024               # PSUM pair width (2 banks)
PAIRS = [(i * PAIR, PAIR) for i in range(VC // PAIR)]
PAIRS.append(((VC // PAIR) * PAIR, VC % PAIR))        # (6144, 106)
NP = len(PAIRS)
# exp pieces [0,3072), [3072,5120), [5120,VC): emitted as soon as the
# underlying pairs land so the last piece (~1.2us) is all that trails
EXP_CUTS = [(0, 3072), (3072, 5120), (5120, VC)]


def build_nc(debug: bool = False):
    nc = bacc.Bacc(
        "TRN2", target_bir_lowering=False, debug=debug, num_devices=NCORES
    )
    # W chunk-tiled + ht tile-major, both per-partition-contiguous in
    # DoubleRow order (contraction row = (2*kk+t)*128+p) -> every load is
    # 128 x one contiguous run (fast HWDGE issue, line-rate transfer)
    wt_d = nc.dram_tensor("wt", [NWCH * 128, KD * 2 * CHUNK], FP8, kind="ExternalInput")
    ht_d = nc.dram_tensor("ht", [128, NBT * KD * 2 * 128], FP8, kind="ExternalInput")
    b_d = nc.dram_tensor("bias", [128, VC], BF16, kind="ExternalInput")
    m4_d = nc.dram_tensor("m4", [128, 1], FP32, kind="ExternalInput")
    out_d = nc.dram_tensor("out", [B, VC], BF16, kind="ExternalOutput")
    sout_d = nc.dram_tensor("sout", [128, 3, NBT], FP32, kind="ExternalOutput")

    with tile.TileContext(nc) as tc:
        with (
            tc.tile_pool(name="const", bufs=1) as const,
            tc.tile_pool(name="mlp", bufs=4) as mlp,
            tc.tile_pool(name="expp", bufs=2) as expp,
            tc.tile_pool(name="ps", bufs=4, space="PSUM") as psp,
        ):
            # ---- PE pre-warm: ~10us of dummy matmuls during the head
            # DMA wait flips the HAM clock gate to 8/8 (2.4 GHz) before
            # the real stream starts and keeps it there (idle gap to the
            # first real MM stays under the ~3.4us re-throttle window)
            scr = const.tile([128, 512], FP8, tag="scr", name="scr")
            nc.vector.memset(scr[:, :], 0.25)
            ps_w = psp.tile([128, PAIR], FP32, tag="ps", name="ps_w")
            for _ in range(38):
                nc.tensor.matmul(
                    ps_w[:, 0:512], lhsT=scr[:, 0:128], rhs=scr[:, :],
                    start=True, stop=True,
                )

            # ---- resident tensors; order = DMA issue priority ---------
            wt_sb = const.tile([128, NWCH, KD, 2, CHUNK], FP8, tag="wt", name="wt_sb")
            ht_sb = const.tile([128, NBT, KD, 2, 128], FP8, tag="ht", name="ht_sb")
            b_sb = const.tile([128, VC], BF16, tag="bias", name="b_sb")

            def dma_w_chunk(ci):
                nc.sync.dma_start(
                    wt_sb[:, ci, :, :, :],
                    wt_d.ap()[ci * 128 : (ci + 1) * 128, :],
                )

            HT1 = KD * 2 * 128    # bytes per tile of ht per partition
            nc.scalar.dma_start(                     # ht for tiles 0,1
                ht_sb[:, 0:2, :, :, :], ht_d.ap()[:, 0 : 2 * HT1]
            )
            dma_w_chunk(0)
            dma_w_chunk(1)
            nc.scalar.dma_start(                     # ht for tiles 2..7
                ht_sb[:, 2:NBT, :, :, :], ht_d.ap()[:, 2 * HT1 :]
            )
            for p in range(NP):
                p0, pw = PAIRS[p]
                nc.scalar.dma_start(
                    b_sb[:, p0 : p0 + pw], b_d.ap()[:, p0 : p0 + pw]
                )
                for ci in (2 * p + 2, 2 * p + 3):
                    if ci < NWCH:
                        dma_w_chunk(ci)
            m4_sb = const.tile([128, 1], FP32, tag="m4", name="m4_sb")
            nc.scalar.dma_start(m4_sb[:, :], m4_d.ap())

            pse = [
                const.tile([128, NBT], FP32, tag=f"pse{k}", name=f"pse{k}")
                for k in range(len(EXP_CUTS))
            ]
            ccin = const.tile([128, 3, NBT], FP32, tag="ccin", name="ccin")
            t1 = const.tile([128, NBT], FP32, tag="t1", name="t1")
            t2 = const.tile([128, NBT], FP32, tag="t2", name="t2")

            ml = {}
            expt = {}

            def mm_pair(j, p):
                """Matmul one 1024-col pair of batch tile j + bias-add to bf16."""
                p0, pw = PAIRS[p]
                if p == 0:
                    ml[j] = mlp.tile([128, VC], BF16, tag="ml", name=f"ml{j}")
                ps = psp.tile([128, PAIR], FP32, tag="ps", name="ps")
                subs = [(0, CHUNK), (CHUNK, pw - CHUNK)] if pw > CHUNK else [(0, pw)]
                # kk outer: consecutive matmuls share the stationary ht
                # slice, halving LDWEIGHTS pressure on the weight path
                for kk in range(KD):
                    for si, (s0, sw) in enumerate(subs):
                        ci = 2 * p + si
                        nc.tensor.matmul(
                            ps[:, s0 : s0 + sw],
                            lhsT=ht_sb[:, j, kk, :, :],
                            rhs=wt_sb[:, ci, kk, :, 0:sw],
                            start=(kk == 0),
                            stop=(kk == KD - 1),
                            perf_mode=mybir.MatmulPerfMode.DoubleRow,
                        )
                nc.vector.tensor_add(
                    ml[j][:, p0 : p0 + pw], ps[:, :pw], b_sb[:, p0 : p0 + pw]
                )

            def out_span(j, lo, hi):
                nc.sync.dma_start(
                    out_d.ap()[j * 128 : (j + 1) * 128, lo:hi], ml[j][:, lo:hi]
                )

            def exp_piece(j, k):
                """exp over one row piece -> partial softmax sum accumulator."""
                lo, hi = EXP_CUTS[k]
                if k == 0:
                    et = expp.tile([128, VC], BF16, tag="exp", name=f"exp{j}")
                    expt[j] = et
                else:
                    et = expt[j]
                nc.scalar.activation(
                    et[:, lo:hi],
                    ml[j][:, lo:hi],
                    AF.Exp,
                    accum_out=pse[k][:, j : j + 1],
                )

            def stats_early(j):
                """Stat lanes that need only the first exp piece (cols 0,4).
                lanes: 1: exp(-l4)*m4    2: e0*m4; prep m4*(1-exp(ml4)) in t2.
                Tiny per-partition ops ride the otherwise-idle ACT engine."""
                et = expt[j]
                tj1, tj2 = t1[:, j : j + 1], t2[:, j : j + 1]
                nc.scalar.activation(
                    tj1, et[:, COPY : COPY + 1], AF.Copy, scale=-1.0, bias=1.0
                )
                nc.scalar.activation(tj2, tj1, AF.Copy, scale=m4_sb[:, :])
                nc.scalar.activation(
                    tj1, ml[j][:, COPY : COPY + 1], AF.Exp, scale=-1.0
                )
                nc.scalar.activation(
                    ccin[:, 1, j : j + 1], tj1, AF.Copy, scale=m4_sb[:, :]
                )
                nc.scalar.activation(
                    ccin[:, 2, j : j + 1],
                    et[:, PAD : PAD + 1],
                    AF.Copy,
                    scale=m4_sb[:, :],
                )

            def stats_late(j):
                """lane0 = sum of exp-piece accumulators + m4*(1-exp(ml4))."""
                tj1 = t1[:, j : j + 1]
                nc.vector.tensor_add(
                    tj1, pse[0][:, j : j + 1], pse[1][:, j : j + 1]
                )
                nc.vector.tensor_add(tj1, tj1, pse[2][:, j : j + 1])
                nc.vector.tensor_add(ccin[:, 0, j : j + 1], tj1, t2[:, j : j + 1])
                nc.sync.dma_start(
                    sout_d.ap()[:, :, j : j + 1], ccin[:, :, j : j + 1]
                )

            # ---------------- emission schedule ------------------------
            # out DMAs: 3 per tile after pairs 1, 3, 6 (keeps the sync
            # queue light); exp pieces after pairs 2, 4, 6
            def tile_step(j, p):
                mm_pair(j, p)
                if p == 1:
                    out_span(j, 0, 2048)
                elif p == 3:
                    out_span(j, 2048, 4096)
                elif p == 6:
                    out_span(j, 4096, VC)
                if p == 2:
                    exp_piece(j, 0)
                elif p == 4:
                    exp_piece(j, 1)

            # phase A: tiles 0,1 pair-outer (chases the W-chunk DMAs)
            for p in range(NP):
                for j in (0, 1):
                    tile_step(j, p)
                if p == 3:
                    stats_early(0)
                    stats_early(1)
            for j in (0, 1):
                exp_piece(j, 2)
                stats_late(j)

            # phase B: remaining tiles tile-outer
            for j in range(2, NBT):
                for p in range(NP):
                    tile_step(j, p)
                    if p == 3:
                        stats_early(j)
                exp_piece(j, 2)
                stats_late(j)

    nc.compile()
    return nc


def prep_inputs(hidden, src, attn, W, b, alignment):
    """Host-side sharding/layout prep. Returns per-core in_maps."""
    bf16 = ml_dtypes.bfloat16
    f8 = ml_dtypes.float8_e4m3
    hidden = np.asarray(hidden, dtype=np.float32)
    W = np.asarray(W, dtype=np.float32)
    b = np.asarray(b, dtype=np.float32)

    # ht tile-major, per-partition-contiguous DoubleRow order:
    # [p, j, kk, t, 128] with contraction row (2*kk+t)*128+p
    htq = hidden.astype(f8).T                                   # [H, B]
    ht = np.ascontiguousarray(
        htq.reshape(KD, 2, 128, NBT, 128)
        .transpose(2, 3, 0, 1, 4)
        .reshape(128, NBT * KD * 2 * 128)
    )
    Wq = W.astype(f8)
    b_bf = b.astype(bf16)

    def pack_w(wcore):
        # wcore [VC, H] -> chunk-tiled [NWCH*128, KD*2*CHUNK], padded
        whv = wcore.T.reshape(KD, 2, 128, VC)                   # [a,t,p,v]
        wp = np.zeros((KD, 2, 128, NWCH * CHUNK), dtype=wcore.dtype)
        wp[..., :VC] = whv
        return np.ascontiguousarray(
            wp.reshape(KD, 2, 128, NWCH, CHUNK)
            .transpose(3, 2, 0, 1, 4)
            .reshape(NWCH * 128, KD * 2 * CHUNK)
        )

    in_maps = []
    for c in range(NCORES):
        vlo, vhi = c * VC, (c + 1) * VC
        m4 = np.full((128, 1), 1.0 if c == 0 else 0.0, np.float32)
        in_maps.append(
            {
                "wt": pack_w(Wq[vlo:vhi, :]),
                "ht": ht,
                "bias": np.ascontiguousarray(
                    np.broadcast_to(b_bf[vlo:vhi][None, :], (128, VC))
                ),
                "m4": m4,
            }
        )
    return in_maps


def postprocess(res, src, attn, alignment):
    """Host: reduce per-core stats, apply per-row C during fp32 convert,
    merge the scatter corrections + PAD/COPY columns."""
    f32 = np.float32
    out = np.concatenate(
        [res.results[c]["out"].astype(f32) for c in range(NCORES)], axis=1
    )
    sall = sum(res.results[c]["sout"].astype(f32) for c in range(NCORES))

    se = sall[:, 0, :].T.reshape(B)              # [NBT,128] -> [B]
    l4e = sall[:, 1, :].T.reshape(B)
    e0 = sall[:, 2, :].T.reshape(B)

    src = np.asarray(src).astype(np.int64)
    alignment = np.asarray(alignment).astype(np.int64)
    attn = np.asarray(attn, dtype=f32)

    copy = (1.0 / (l4e + 1.0)).astype(f32)
    omc = (1.0 - copy).astype(f32)
    tgt = alignment[src]
    anz = (attn * (tgt != PAD)).sum(axis=1).astype(f32)
    norm = (omc * (1.0 - e0 / se) + copy * anz + EPS).astype(f32)
    C = np.log(omc / (se * norm)).astype(f32)

    out += C[:, None]

    D = (copy / norm).astype(f32)
    a4 = (omc / (se * norm)).astype(f32)

    out[:, COPY] = np.log(a4 + EPS)

    rows = np.repeat(np.arange(B), S)
    keys = rows * V + tgt.ravel()
    uk, inv = np.unique(keys, return_inverse=True)
    acc = np.bincount(inv, weights=attn.ravel().astype(np.float64)).astype(f32)
    ub = (uk // V).astype(np.int64)
    uv = (uk % V).astype(np.int64)
    m = uv != PAD
    ubm, uvm, accm = ub[m], uv[m], acc[m]
    base_arg = np.where(uvm == COPY, a4[ubm], np.exp(out[ubm, uvm]))
    out[ubm, uvm] = np.log(base_arg + D[ubm] * accm + EPS)

    out[:, PAD] = np.log(EPS / norm + EPS)
    return out


_NC_CACHE = {}


def _get_nc(debug=False):
    key = bool(debug)
    if key not in _NC_CACHE:
        _NC_CACHE[key] = build_nc(debug=debug)
    return _NC_CACHE[key]


def run(inputs, trace=False):
    """Run on hardware; returns (full_output, BassKernelResults)."""
    nc = _get_nc()
    in_maps = prep_inputs(**inputs)
    res = bass_utils.run_bass_kernel_spmd(
        nc, in_maps, core_ids=list(range(NCORES)), trace=trace
    )
    out = postprocess(res, inputs["src"], inputs["attn"], inputs["alignment"])
    return out, res


def kernel(**inputs) -> np.ndarray:
    out, _ = run(inputs, trace=False)
    return out


# revision 31
# speedup vs baseline: 1.0366x; 1.0366x over previous
# CopyGenerator kernel for 8 TRN2 NeuronCores (Bass/Tile, SPMD).
#
# reference computation:
#   logits = hidden @ W.T + b                      [B=1024, V=50000]
#   ml = logits with col COPY(4) = 1e-10
#   prob = softmax(ml); copy = sigmoid(logits[:, 4])
#   out_prob = prob*(1-copy); out_prob[b, alignment[src[b,s]]] += attn[b,s]*copy[b]
#   out_prob[:, 0] = EPS; norm = out_prob.sum(-1)
#   out = log(out_prob/norm + EPS)
#
# Strategy (tensor-parallel over vocab, VC=6250 cols/core): for the
# ~49.9k/50k columns with no scatter contribution,
#   out[b,v] = ml[b,v] + C[b],  C = ln((1-copy)/(se*norm))
# exactly (log-domain identity; inner +EPS is negligible: norm-rel ~9e-5,
# validated in simcheck.py).  Device per core: fp8 DoubleRow matmul ->
# DVE adds bias & casts bf16 (1024-wide PSUM pairs) -> bf16 out DMA per
# span -> ACT exp in 3 pieces per row, emitted as the underlying pairs
# land (accum_out -> per-row partial softmax sums) -> ships a tiny
# [128,3,8] per-row stats tensor.  The host sums the 8 cores' partial
# stats (24KB total), forms C, and adds it during the bf16->fp32
# conversion, then patches the <=128 scattered columns per row + the
# PAD/COPY columns.  No collectives: on this axon setup the entry
# barrier + AllReduce cost 40-60us (cores start skewed), which starved
# the in-order DVE queue and bunched all output DMA into the tail.
#
# Per-core HBM: W 6.8MB (resident, streamed once) + ht 1MB + bias 1.6MB
# + out 12.8MB bf16 ~= 22MB.  TensorE is the bottleneck and runs at the
# fp8-DoubleRow roofline: 416 matmuls at ~217ns each (N=512 streaming
# rate), gap-free between the dummy-warmup head and the 3-piece exp tail.
import numpy as np
import ml_dtypes

import concourse.bacc as bacc
import concourse.bass as bass
import concourse.mybir as mybir
import concourse.tile as tile
from concourse import bass_utils

FP32 = mybir.dt.float32
BF16 = mybir.dt.bfloat16
FP8 = mybir.dt.float8e4
AF = mybir.ActivationFunctionType
ALU = mybir.AluOpType

B, S, H, V = 1024, 128, 1024, 50000
NCORES = 8
VC = V // NCORES          # 6250 vocab columns per core
NBT = B // 128            # 8 batch tiles of 128 rows
KD = 4                    # 4 DoubleRow chunks of K=256
COPY, PAD, EPS = 4, 0, 1e-10

CHUNK = 512               # W-DMA granularity (last chunk zero-padded)
NWCH = 13                 # 12x512 + 106
PAIR = 1024               # PSUM pair width (2 banks)
PAIRS = [(i * PAIR, PAIR) for i in range(VC // PAIR)]
PAIRS.append(((VC // PAIR) * PAIR, VC % PAIR))        # (6144, 106)
NP = len(PAIRS)
# exp pieces [0,3072), [3072,5120), [5120,VC): emitted as soon as the
# underlying pairs land so the last piece (~1.2us) is all that trails
EXP_CUTS = [(0, 3072), (3072, 5120), (5120, VC)]


def build_nc(debug: bool = False):
    nc = bacc.Bacc(
        "TRN2", target_bir_lowering=False, debug=debug, num_devices=NCORES
    )
    # W chunk-tiled + ht tile-major, both per-partition-contiguous in
    # DoubleRow order (contraction row = (2*kk+t)*128+p) -> every load is
    # 128 x one contiguous run (fast HWDGE issue, line-rate transfer)
    wt_d = nc.dram_tensor("wt", [NWCH * 128, KD * 2 * CHUNK], FP8, kind="ExternalInput")
    ht_d = nc.dram_tensor("ht", [128, NBT * KD * 2 * 128], FP8, kind="ExternalInput")
    b_d = nc.dram_tensor("bias", [128, VC], BF16, kind="ExternalInput")
    m4_d = nc.dram_tensor("m4", [128, 1], FP32, kind="ExternalInput")
    out_d = nc.dram_tensor("out", [B, VC], BF16, kind="ExternalOutput")
    sout_d = nc.dram_tensor("sout", [128, 3, NBT], FP32, kind="ExternalOutput")

    with tile.TileContext(nc) as tc:
        with (
            tc.tile_pool(name="const", bufs=1) as const,
            tc.tile_pool(name="mlp", bufs=4) as mlp,
            tc.tile_pool(name="expp", bufs=2) as expp,
            tc.tile_pool(name="ps", bufs=4, space="PSUM") as psp,
        ):
            # ---- PE pre-warm: ~10us of dummy matmuls during the head
            # DMA wait flips the HAM clock gate to 8/8 (2.4 GHz) before
            # the real stream starts and keeps it there (idle gap to the
            # first real MM stays under the ~3.4us re-throttle window)
            scr = const.tile([128, 512], FP8, tag="scr", name="scr")
            nc.vector.memset(scr[:, :], 0.25)
            ps_w = psp.tile([128, PAIR], FP32, tag="ps", name="ps_w")
            for _ in range(38):
                nc.tensor.matmul(
                    ps_w[:, 0:512], lhsT=scr[:, 0:128], rhs=scr[:, :],
                    start=True, stop=True,
                )

            # ---- resident tensors; order = DMA issue priority ---------
            wt_sb = const.tile([128, NWCH, KD, 2, CHUNK], FP8, tag="wt", name="wt_sb")
            ht_sb = const.tile([128, NBT, KD, 2, 128], FP8, tag="ht", name="ht_sb")
            b_sb = const.tile([128, VC], BF16, tag="bias", name="b_sb")

            def dma_w_chunk(ci):
                nc.sync.dma_start(
                    wt_sb[:, ci, :, :, :],
                    wt_d.ap()[ci * 128 : (ci + 1) * 128, :],
                )

            HT1 = KD * 2 * 128    # bytes per tile of ht per partition
            dma_w_chunk(0)
            nc.sync.dma_start(                       # ht for tiles 0,1
                ht_sb[:, 0:2, :, :, :], ht_d.ap()[:, 0 : 2 * HT1]
            )
            dma_w_chunk(1)
            nc.sync.dma_start(                       # ht for tiles 2..7
                ht_sb[:, 2:NBT, :, :, :], ht_d.ap()[:, 2 * HT1 :]
            )
            for p in range(NP):
                p0, pw = PAIRS[p]
                nc.sync.dma_start(
                    b_sb[:, p0 : p0 + pw], b_d.ap()[:, p0 : p0 + pw]
                )
                for ci in (2 * p + 2, 2 * p + 3):
                    if ci < NWCH:
                        dma_w_chunk(ci)
            m4_sb = const.tile([128, 1], FP32, tag="m4", name="m4_sb")
            nc.sync.dma_start(m4_sb[:, :], m4_d.ap())

            pse = [
                const.tile([128, NBT], FP32, tag=f"pse{k}", name=f"pse{k}")
                for k in range(len(EXP_CUTS))
            ]
            ccin = const.tile([128, 3, NBT], FP32, tag="ccin", name="ccin")
            t1 = const.tile([128, NBT], FP32, tag="t1", name="t1")
            t2 = const.tile([128, NBT], FP32, tag="t2", name="t2")

            ml = {}
            expt = {}

            def mm_pair(j, p):
                """Matmul one 1024-col pair of batch tile j + bias-add to bf16."""
                p0, pw = PAIRS[p]
                if p == 0:
                    ml[j] = mlp.tile([128, VC], BF16, tag="ml", name=f"ml{j}")
                ps = psp.tile([128, PAIR], FP32, tag="ps", name="ps")
                subs = [(0, CHUNK), (CHUNK, pw - CHUNK)] if pw > CHUNK else [(0, pw)]
                # kk outer: consecutive matmuls share the stationary ht
                # slice, halving LDWEIGHTS pressure on the weight path
                for kk in range(KD):
                    for si, (s0, sw) in enumerate(subs):
                        ci = 2 * p + si
                        nc.tensor.matmul(
                            ps[:, s0 : s0 + sw],
                            lhsT=ht_sb[:, j, kk, :, :],
                            rhs=wt_sb[:, ci, kk, :, 0:sw],
                            start=(kk == 0),
                            stop=(kk == KD - 1),
                            perf_mode=mybir.MatmulPerfMode.DoubleRow,
                        )
                nc.vector.tensor_add(
                    ml[j][:, p0 : p0 + pw], ps[:, :pw], b_sb[:, p0 : p0 + pw]
                )

            def out_span(j, lo, hi):
                nc.sync.dma_start(
                    out_d.ap()[j * 128 : (j + 1) * 128, lo:hi], ml[j][:, lo:hi]
                )

            def exp_piece(j, k):
                """exp over one row piece -> partial softmax sum accumulator."""
                lo, hi = EXP_CUTS[k]
                if k == 0:
                    et = expp.tile([128, VC], BF16, tag="exp", name=f"exp{j}")
                    expt[j] = et
                else:
                    et = expt[j]
                nc.scalar.activation(
                    et[:, lo:hi],
                    ml[j][:, lo:hi],
                    AF.Exp,
                    accum_out=pse[k][:, j : j + 1],
                )

            def stats_early(j):
                """Stat lanes that need only the first exp piece (cols 0,4).
                lanes: 1: exp(-l4)*m4    2: e0*m4; prep m4*(1-exp(ml4)) in t2.
                Tiny per-partition ops ride the otherwise-idle ACT engine."""
                et = expt[j]
                tj1, tj2 = t1[:, j : j + 1], t2[:, j : j + 1]
                nc.scalar.activation(
                    tj1, et[:, COPY : COPY + 1], AF.Copy, scale=-1.0, bias=1.0
                )
                nc.scalar.activation(tj2, tj1, AF.Copy, scale=m4_sb[:, :])
                nc.scalar.activation(
                    tj1, ml[j][:, COPY : COPY + 1], AF.Exp, scale=-1.0
                )
                nc.scalar.activation(
                    ccin[:, 1, j : j + 1], tj1, AF.Copy, scale=m4_sb[:, :]
                )
                nc.scalar.activation(
                    ccin[:, 2, j : j + 1],
                    et[:, PAD : PAD + 1],
                    AF.Copy,
                    scale=m4_sb[:, :],
                )

            def stats_late(j):
                """lane0 = sum of exp-piece accumulators + m4*(1-exp(ml4))."""
                tj1 = t1[:, j : j + 1]
                nc.vector.tensor_add(
                    tj1, pse[0][:, j : j + 1], pse[1][:, j : j + 1]
                )
                nc.vector.tensor_add(tj1, tj1, pse[2][:, j : j + 1])
                nc.vector.tensor_add(ccin[:, 0, j : j + 1], tj1, t2[:, j : j + 1])
                nc.sync.dma_start(
                    sout_d.ap()[:, :, j : j + 1], ccin[:, :, j : j + 1]
                )

            # ---------------- emission schedule ------------------------
            # out DMAs: 3 per tile after pairs 1, 3, 6 (keeps the sync
            # queue light); exp pieces after pairs 2, 4, 6
            def tile_step(j, p):
                mm_pair(j, p)
                if p == 1:
                    out_span(j, 0, 2048)
                elif p == 3:
                    out_span(j, 2048, 4096)
                elif p == 6:
                    out_span(j, 4096, VC)
                if p == 2:
                    exp_piece(j, 0)
                elif p == 4:
                    exp_piece(j, 1)

            # phase A: tiles 0,1 pair-outer (chases the W-chunk DMAs)
            for p in range(NP):
                for j in (0, 1):
                    tile_step(j, p)
                if p == 3:
                    stats_early(0)
                    stats_early(1)
            for j in (0, 1):
                exp_piece(j, 2)
                stats_late(j)

            # phase B: remaining tiles tile-outer
            for j in range(2, NBT):
                for p in range(NP):
                    tile_step(j, p)
                    if p == 3:
                        stats_early(j)
                exp_piece(j, 2)
                stats_late(j)

    nc.compile()
    return nc


def prep_inputs(hidden, src, attn, W, b, alignment):
    """Host-side sharding/layout prep. Returns per-core in_maps."""
    bf16 = ml_dtypes.bfloat16
    f8 = ml_dtypes.float8_e4m3
    hidden = np.asarray(hidden, dtype=np.float32)
    W = np.asarray(W, dtype=np.float32)
    b = np.asarray(b, dtype=np.float32)

    # ht tile-major, per-partition-contiguous DoubleRow order:
    # [p, j, kk, t, 128] with contraction row (2*kk+t)*128+p
    htq = hidden.astype(f8).T                                   # [H, B]
    ht = np.ascontiguousarray(
        htq.reshape(KD, 2, 128, NBT, 128)
        .transpose(2, 3, 0, 1, 4)
        .reshape(128, NBT * KD * 2 * 128)
    )
    Wq = W.astype(f8)
    b_bf = b.astype(bf16)

    def pack_w(wcore):
        # wcore [VC, H] -> chunk-tiled [NWCH*128, KD*2*CHUNK], padded
        whv = wcore.T.reshape(KD, 2, 128, VC)                   # [a,t,p,v]
        wp = np.zeros((KD, 2, 128, NWCH * CHUNK), dtype=wcore.dtype)
        wp[..., :VC] = whv
        return np.ascontiguousarray(
            wp.reshape(KD, 2, 128, NWCH, CHUNK)
            .transpose(3, 2, 0, 1, 4)
            .reshape(NWCH * 128, KD * 2 * CHUNK)
        )

    in_maps = []
    for c in range(NCORES):
        vlo, vhi = c * VC, (c + 1) * VC
        m4 = np.full((128, 1), 1.0 if c == 0 else 0.0, np.float32)
        in_maps.append(
            {
                "wt": pack_w(Wq[vlo:vhi, :]),
                "ht": ht,
                "bias": np.ascontiguousarray(
                    np.broadcast_to(b_bf[vlo:vhi][None, :], (128, VC))
                ),
                "m4": m4,
            }
        )
    return in_maps


def postprocess(res, src, attn, alignment):
    """Host: reduce per-core stats, apply per-row C during fp32 convert,
    merge the scatter corrections + PAD/COPY columns."""
    f32 = np.float32
    out = np.concatenate(
        [res.results[c]["out"].astype(f32) for c in range(NCORES)], axis=1
    )
    sall = sum(res.results[c]["sout"].astype(f32) for c in range(NCORES))

    se = sall[:, 0, :].T.reshape(B)              # [NBT,128] -> [B]
    l4e = sall[:, 1, :].T.reshape(B)
    e0 = sall[:, 2, :].T.reshape(B)

    src = np.asarray(src).astype(np.int64)
    alignment = np.asarray(alignment).astype(np.int64)
    attn = np.asarray(attn, dtype=f32)

    copy = (1.0 / (l4e + 1.0)).astype(f32)
    omc = (1.0 - copy).astype(f32)
    tgt = alignment[src]
    anz = (attn * (tgt != PAD)).sum(axis=1).astype(f32)
    norm = (omc * (1.0 - e0 / se) + copy * anz + EPS).astype(f32)
    C = np.log(omc / (se * norm)).astype(f32)

    out += C[:, None]

    D = (copy / norm).astype(f32)
    a4 = (omc / (se * norm)).astype(f32)

    out[:, COPY] = np.log(a4 + EPS)

    rows = np.repeat(np.arange(B), S)
    keys = rows * V + tgt.ravel()
    uk, inv = np.unique(keys, return_inverse=True)
    acc = np.bincount(inv, weights=attn.ravel().astype(np.float64)).astype(f32)
    ub = (uk // V).astype(np.int64)
    uv = (uk % V).astype(np.int64)
    m = uv != PAD
    ubm, uvm, accm = ub[m], uv[m], acc[m]
    base_arg = np.where(uvm == COPY, a4[ubm], np.exp(out[ubm, uvm]))
    out[ubm, uvm] = np.log(base_arg + D[ubm] * accm + EPS)

    out[:, PAD] = np.log(EPS / norm + EPS)
    return out


_NC_CACHE = {}


def _get_nc(debug=False):
    key = bool(debug)
    if key not in _NC_CACHE:
        _NC_CACHE[key] = build_nc(debug=debug)
    return _NC_CACHE[key]


def run(inputs, trace=False):
    """Run on hardware; returns (full_output, BassKernelResults)."""
    nc = _get_nc()
    in_maps = prep_inputs(**inputs)
    res = bass_utils.run_bass_kernel_spmd(
        nc, in_maps, core_ids=list(range(NCORES)), trace=trace
    )
    out = postprocess(res, inputs["src"], inputs["attn"], inputs["alignment"])
    return out, res


def kernel(**inputs) -> np.ndarray:
    out, _ = run(inputs, trace=False)
    return out
